# revision 18
# baseline (speedup 1.0000x reference)
"""Trainium2 Bass kernel for nn_ComplexMixture.

Reference:
  output_real[b,n,m] = sum_s w[b,s] * (r[b,s,n]*r[b,s,m] + i[b,s,n]*i[b,s,m])
  output_imag[b,n,m] = sum_s w[b,s] * (i[b,s,n]*r[b,s,m] - r[b,s,n]*i[b,s,m])

Shapes: B=32, S=128, N=256, fp32. w is uniform [0,1) so sqrt(w) is real.

out_r is symmetric and out_i is antisymmetric, so the device only computes
  P = out_r + out_i
and the host recovers out_r = (P + P^T)/2, out_i = (P - P^T)/2.
The host pre-scales the inputs: Yr = sqrt(w)[:,None]*r, Yi = sqrt(w)[:,None]*i.
With U = Yr - Yi, V = Yr + Yi:
  P[n,m] = sum_s Yr[s,n]*U[s,m] + Yi[s,n]*V[s,m]
i.e. per 128-row output chunk c:  P_c = Yr_c.T @ U + Yi_c.T @ V  (PSUM accum).

Final version (19.5us vs 24.5us baseline; measured plateau of 8 variants):
  - bf16 end-to-end: bf16 inputs/outputs halve both DMA directions (input
    reads are the critical stream at ~230-300 GB/s aggregate), bf16 matmuls
    run 1 cycle/row, DVE gets 2x mode. rel err 4.3e-3 vs the 2e-2 gate
    (PSUM accumulation stays fp32; error matches the host bf16 simulation).
  - 3 input kicks, one per queue (sync/scalar HWDGE rings get pair0 split by
    partition halves; gpsimd SWDGE carries pair1), all 2KB packets, issued
    as the first body instructions. First use of a DMA queue costs ~0.8-1.6us
    start latency; outputs reuse the warmed queues (~0.4us pickup).
  - small PE warmup keeps the tensor engine fed while inputs stream. (The
    HAM clock gate needs ~9us of sustained PE activity to release on this
    part, so a ~20us kernel runs all matmuls at 1.2 GHz: 213ns per 256-wide
    matmul. 16 real matmuls = 3.4us, which paces the middle section.)
  - per-batch UV on vector, PSUM->SBUF bf16 casts alternating vector/scalar,
    per-batch output DMAs pipelined on hot queues; the tail batch drains as
    two independent cast->kick chains on two queues (no cross-engine join).

Measured structure (exec ~= last-output-byte + 2.7us of fixed drain/barrier):
  0-6.7us framework preamble | 6.7-7.5 kicks | 8.2-10.1 input streams |
  10.1-14.1 UV+matmul pipeline | 12.1-16.9 casts + output streams.

Data-parallel over B across 8 cores, 4 batches/core:
  xpack [S, BPC*2*N] bf16: per partition s: [b0:(Yr|Yi) | b1:(Yr|Yi) | ...]
  out   [128, BPC*2*N] bf16: per partition p: [b][c][m] -> P[b, c*128+p, m].
"""

import os

import numpy as np

import concourse.bass as bass
import concourse.mybir as mybir
import concourse.tile as tile
from concourse import bacc
from concourse.bass_utils import run_bass_kernel_spmd

B, S, N = 32, 128, 256
NCORES = 8
BPC = B // NCORES  # batches per core
W = 2 * N  # columns per batch block
XCOL = BPC * W  # 2048 bf16 per partition row

F32 = mybir.dt.float32
BF16 = mybir.dt.bfloat16

N_WARMUP = int(os.environ.get("CM_WARMUP", "8"))

LAST_RESULTS = None  # stashed BassKernelResults for test harness introspection


def build_nc() -> bass.Bass:
    nc = bacc.Bacc(num_swdge_queues=1)
    xin = nc.dram_tensor("xpack", [S, XCOL], BF16, kind="ExternalInput")
    out = nc.dram_tensor("out_all", [128, XCOL], BF16, kind="ExternalOutput")

    with tile.TileContext(nc) as tc:
        with (
            tc.tile_pool(name="io", bufs=1) as io_pool,
            tc.tile_pool(name="uv", bufs=1) as uv_pool,
            tc.tile_pool(name="op", bufs=1) as out_pool,
            tc.tile_pool(name="ps", bufs=BPC, space="PSUM") as ps_pool,
        ):
            X_all = io_pool.tile([S, XCOL], BF16, tag="X", name="X_all")

            # Input DMAs first: pair0 = b0b1 split by partition halves on the
            # two HWDGE rings (2KB packets), b2 as second HWDGE kicks, and
            # only b3 on SWDGE — the SW queue's stream time varies 1.1-2.0us
            # per 256KB across runs, so it gets the smallest share.
            nc.sync.dma_start(out=X_all[0:64, 0 : 2 * W], in_=xin[0:64, 0 : 2 * W])
            nc.scalar.dma_start(out=X_all[64:128, 0 : 2 * W], in_=xin[64:128, 0 : 2 * W])
            nc.gpsimd.dma_start(out=X_all[:, 3 * W : 4 * W], in_=xin[:, 3 * W : 4 * W])
            nc.sync.dma_start(out=X_all[0:64, 2 * W : 3 * W], in_=xin[0:64, 2 * W : 3 * W])
            nc.scalar.dma_start(out=X_all[64:128, 2 * W : 3 * W], in_=xin[64:128, 2 * W : 3 * W])

            # Single UV tile for all batches (fewer tiles -> fewer teardown
            # semaphore resets inside the exec window).
            UVa = uv_pool.tile([S, XCOL], BF16, tag="UV", name="UV_all")
            O = out_pool.tile([128, XCOL], BF16, tag="O", name="O_all")
            for b in range(BPC):
                X = X_all[:, b * W : (b + 1) * W]
                Yr = X[:, 0:N]
                Yi = X[:, N:W]
                UV = UVa[:, b * W : (b + 1) * W]
                nc.vector.tensor_sub(UV[:, 0:N], Yr, Yi)
                nc.vector.tensor_add(UV[:, N:W], Yr, Yi)

                ps = ps_pool.tile([128, W], F32, tag="ps", name=f"ps{b}")
                for c in range(2):
                    csl = slice(c * 128, c * 128 + 128)
                    osl = slice(c * N, (c + 1) * N)
                    nc.tensor.matmul(ps[:, osl], lhsT=Yr[:, csl], rhs=UV[:, 0:N], start=True, stop=False)
                    nc.tensor.matmul(ps[:, osl], lhsT=Yi[:, csl], rhs=UV[:, N:W], start=False, stop=True)

                o0 = slice(b * W, b * W + N)
                o1 = slice(b * W + N, (b + 1) * W)
                osl_all = slice(b * W, (b + 1) * W)
                if b == BPC - 1:
                    # Tail batch: two independent cast->kick chains. The
                    # scalar half self-kicks (no cross-engine join); sync
                    # kicks the vector half.
                    nc.vector.tensor_copy(O[:, o0], ps[:, 0:N])
                    nc.sync.dma_start(out=out[:, o0], in_=O[:, o0])
                    nc.scalar.copy(out=O[:, o1], in_=ps[:, N:W])
                    nc.scalar.dma_start(out=out[:, o1], in_=O[:, o1])
                else:
                    if b % 2 == 0:
                        nc.vector.tensor_copy(O[:, osl_all], ps)
                        nc.sync.dma_start(out=out[:, osl_all], in_=O[:, osl_all])
                    else:
                        nc.scalar.copy(out=O[:, osl_all], in_=ps)
                        nc.scalar.dma_start(out=out[:, osl_all], in_=O[:, osl_all])
    nc.compile()
    return nc


def kernel(**inputs: np.ndarray):
    global LAST_RESULTS
    import ml_dtypes

    r = np.asarray(inputs["input_real"], dtype=np.float32)
    i = np.asarray(inputs["input_imag"], dtype=np.float32)
    w = np.ascontiguousarray(np.asarray(inputs["weight"], dtype=np.float32))
    assert r.shape == (B, S, N) and i.shape == (B, S, N) and w.shape == (B, S)

    # [B, 2, S, N] -> per-core [S, (b t n)] batch-major blocks, bf16
    sws = np.sqrt(w)  # [B, S]
    xin = (np.stack([r, i], axis=1) * sws[:, None, :, None]).astype(ml_dtypes.bfloat16)

    in_maps = []
    for c in range(NCORES):
        sl = slice(c * BPC, (c + 1) * BPC)
        xpack = np.transpose(xin[sl], (2, 0, 1, 3)).reshape(S, XCOL)
        in_maps.append({"xpack": np.ascontiguousarray(xpack)})

    nc = build_nc()
    res = run_bass_kernel_spmd(nc, in_maps, core_ids=list(range(NCORES)))
    LAST_RESULTS = res

    # out_all[core] is [128, (b c m)] bf16; P[b, c*128+p, m] = out[p, b*512 + c*256 + m]
    out_all = np.stack(
        [np.asarray(res.results[c]["out_all"]) for c in range(NCORES)], axis=0
    ).astype(np.float32)  # [NCORES, 128, XCOL]
    out_all = out_all.reshape(NCORES, 128, BPC, 2, N)
    P = np.transpose(out_all, (0, 2, 3, 1, 4)).reshape(B, N, N)
    Pt = np.transpose(P, (0, 2, 1))
    out_r = (P + Pt) * np.float32(0.5)
    out_i = (P - Pt) * np.float32(0.5)
    return (np.ascontiguousarray(out_r), np.ascontiguousarray(out_i))
